# revision 1
# baseline (speedup 1.0000x reference)
"""Deformable PS-ROI pooling on Trainium2 (Bass/Tile), SPMD over 8 cores.

Strategy: data-parallel over ROIs (64 rois/core), feature map replicated in
DRAM in channel-last bf16 layout.  The two x-corners of a bilinear sample
are always adjacent pixels (x1, x1+1), so each gather descriptor fetches 2
contiguous pixels (1 KiB); HW gather cost is descriptor-bound, so this
halves gather time vs per-pixel descriptors.  ROIs are processed in pairs
(q, q+32): one 784-descriptor dma_gather per pair (fits the ~1024-desc Q7
idx scratch cap).  Descriptor i = 16*bin + 2*t3 + r (t3 = sample_h x
corner_y x sample_w, r = roi half) lands at partition 16*(bin%8)+2*t3+r,
so a single mask constant works for every column and the 16-lane index
tensor is a plain per-lane DMA.  A [128, 98] bf16 mask matmul on the PE
reduces each landing column into the pair's [98, 256] psum (rows 0..48
roi q, 49..97 roi q+32), with all bilinear / validity / 1-over-count
factors pre-folded into per-partition scalar weights (separate left- and
right-pixel variants, placed by stride-2-partition DMAs).
"""

import numpy as np
import ml_dtypes

import concourse.bass as bass
import concourse.bacc as bacc
import concourse.mybir as mybir
from concourse import tile
from concourse.bass_utils import run_bass_kernel_spmd

F32 = mybir.dt.float32
BF16 = mybir.dt.bfloat16
I32 = mybir.dt.int32
I16 = mybir.dt.int16
OP = mybir.AluOpType

N_CORES = 8
R = 64                  # rois per core
P = 7                   # pooled output size
NB = P * P              # 49 bins
CH = 256                # channels
H = W = 128             # feature map spatial
B = 2                   # batch
NPX = B * H * W         # 32768 flat pixels
PAD = 4                 # extra zero pixels (right-px overrun at x1=W-1)
T3 = 8                  # terms per (bin, roi): sample_h x corner_y x sample_w
TD = NB * T3            # 392 descriptor-terms per roi
ND = 2 * TD             # 784 descriptors per pair
NPAIR = R // 2
NCOL = 7                # gather dest cols per pair (784 = 6*128 + 16)
EL = 2 * CH             # elements per descriptor (2 pixels)
M2 = 2 * NB             # 98 psum rows per pair
SCALE = 0.0625
TRANS_STD = 0.1
GP_BUFS = 8


def _floor(nc, pool, x, name):
    """floor(x) robust to convert rounding mode: returns (floor_f32, frac)."""
    xi = pool.tile([R, x.shape[1]], I32, tag=name + "_i")
    nc.vector.tensor_copy(xi[:, :], x)
    xf = pool.tile([R, x.shape[1]], F32, tag=name + "_f")
    nc.vector.tensor_copy(xf[:, :], xi[:, :])
    d = pool.tile([R, x.shape[1]], F32, tag=name + "_d")
    nc.vector.tensor_tensor(d[:, :], x, xf[:, :], OP.subtract)
    neg = pool.tile([R, x.shape[1]], F32, tag=name + "_n")
    nc.vector.tensor_scalar(neg[:, :], d[:, :], 0.0, None, OP.is_lt)
    fl = pool.tile([R, x.shape[1]], F32, tag=name + "_fl")
    nc.vector.tensor_tensor(fl[:, :], xf[:, :], neg[:, :], OP.subtract)
    fr = pool.tile([R, x.shape[1]], F32, tag=name + "_fr")
    nc.vector.tensor_tensor(fr[:, :], d[:, :], neg[:, :], OP.add)
    return fl[:, :], fr[:, :]


def build_program(reps: int = 1, bench_mode: int = 0):
    """bench_mode: 0=full kernel, 1=gathers only (no reduce), 2=no gathers."""
    nc = bacc.Bacc("TRN2", target_bir_lowering=False, debug=False, num_swdge_queues=4)
    nc.dynamic_dma_scratch_size = 2 ** 16

    data = nc.dram_tensor("data_t", [NPX + PAD, CH], BF16, kind="ExternalInput")
    rois_d = nc.dram_tensor("rois", [R, 5], F32, kind="ExternalInput")
    off_d = nc.dram_tensor("offs", [R, 2 * NB], F32, kind="ExternalInput")
    iopw_d = nc.dram_tensor("iota_pw", [R, NB], F32, kind="ExternalInput")
    ioph_d = nc.dram_tensor("iota_ph", [R, NB], F32, kind="ExternalInput")
    iden_d = nc.dram_tensor("identity", [R, R], F32, kind="ExternalInput")
    cm_d = nc.dram_tensor("cmask", [128, NCOL * M2], F32, kind="ExternalInput")
    out_d = nc.dram_tensor("out", [R, NB * CH], F32, kind="ExternalOutput")

    with tile.TileContext(nc) as tc:
        with (
            tc.tile_pool(name="const", bufs=1) as cst,
            tc.tile_pool(name="work", bufs=1) as wk,
            tc.tile_pool(name="gp", bufs=GP_BUFS) as gp,
            tc.tile_pool(name="gwp", bufs=8) as gwp,
            tc.tile_pool(name="obp", bufs=3) as obp,
            tc.tile_pool(name="psp", bufs=4, space="PSUM") as psp,
            tc.tile_pool(name="pst", bufs=2, space="PSUM") as pst,
        ):
            # ---- load inputs / constants to SBUF ----
            rois = cst.tile([R, 5], F32)
            nc.sync.dma_start(rois[:, :], rois_d.ap())
            off = cst.tile([R, 2 * NB], F32)
            nc.sync.dma_start(off[:, :], off_d.ap())
            iopw = cst.tile([R, NB], F32)
            nc.sync.dma_start(iopw[:, :], iopw_d.ap())
            ioph = cst.tile([R, NB], F32)
            nc.sync.dma_start(ioph[:, :], ioph_d.ap())
            iden = cst.tile([R, R], F32)
            nc.sync.dma_start(iden[:, :], iden_d.ap())
            cm = cst.tile([128, NCOL * M2], F32)
            nc.sync.dma_start(cm[:, :], cm_d.ap())

            # gather source: 2 contiguous pixels per desc, row stride 1 pixel
            a0 = data.ap()
            dap = bass.AP(a0.tensor, a0.offset, [[CH, NPX], [1, EL]])

            from contextlib import nullcontext
            loop_cm = tc.For_i(0, reps, 1) if reps > 1 else nullcontext()
            with loop_cm:
                # ---- phase A: per-roi coordinate math, roi on partition ----
                # round(rois[:,1:5]) = floor(x + 0.5)
                rr = wk.tile([R, 4], F32)
                nc.vector.tensor_scalar(rr[:, :], rois[:, 1:5], 0.5, None, OP.add)
                rnd, _ = _floor(nc, wk, rr[:, :], "rnd")

                # start/end in feature coords
                swsh = wk.tile([R, 2], F32)
                nc.vector.tensor_scalar(swsh[:, :], rnd[:, 0:2], SCALE, -0.5, OP.mult, OP.add)
                eweh = wk.tile([R, 2], F32)
                nc.vector.tensor_scalar(
                    eweh[:, :], rnd[:, 2:4], SCALE, SCALE - 0.5, OP.mult, OP.add
                )
                rwh0 = wk.tile([R, 2], F32)
                nc.vector.tensor_tensor(rwh0[:, :], eweh[:, :], swsh[:, :], OP.subtract)
                rwh = wk.tile([R, 2], F32)
                nc.vector.tensor_scalar(rwh[:, :], rwh0[:, :], 0.1, None, OP.max)
                bwh = wk.tile([R, 2], F32)
                nc.vector.tensor_scalar(bwh[:, :], rwh[:, :], 1.0 / P, None, OP.mult)
                swh = wk.tile([R, 2], F32)
                nc.vector.tensor_scalar(swh[:, :], bwh[:, :], 0.5, None, OP.mult)
                rwh01 = wk.tile([R, 2], F32)
                nc.vector.tensor_scalar(rwh01[:, :], rwh[:, :], TRANS_STD, None, OP.mult)
                ybase = wk.tile([R, 1], F32)
                nc.vector.tensor_scalar(ybase[:, :], rois[:, 0:1], float(H * W), None, OP.mult)

                # bin starts, shifted by learned offsets: [R, 49]
                def bin_start(iota, bcol, scol, tview, r01col, name):
                    t0 = wk.tile([R, NB], F32, tag=name + "0")
                    nc.vector.tensor_scalar(t0[:, :], iota, bcol, None, OP.mult)
                    t1 = wk.tile([R, NB], F32, tag=name + "1")
                    nc.vector.scalar_tensor_tensor(
                        t1[:, :], tview, r01col, t0[:, :], OP.mult, OP.add
                    )
                    t2 = wk.tile([R, NB], F32, tag=name + "2")
                    nc.vector.tensor_scalar(t2[:, :], t1[:, :], scol, None, OP.add)
                    return t2

                wstart = bin_start(
                    iopw[:, :], bwh[:, 0:1], swsh[:, 0:1], off[:, 0:NB],
                    rwh01[:, 0:1], "ws",
                )
                hstart = bin_start(
                    ioph[:, :], bwh[:, 1:2], swsh[:, 1:2], off[:, NB : 2 * NB],
                    rwh01[:, 1:2], "hs",
                )

                # sample positions [R, 98] = (bin, s)
                def samples(start, subcol, name):
                    s2 = wk.tile([R, 2 * NB], F32, tag=name)
                    v = s2[:, :].rearrange("p (b s) -> p b s", s=2)
                    su = start[:, :].rearrange("p b -> p b", ).unsqueeze(2)
                    nc.vector.tensor_copy(v[:, :, 0:1], su)
                    nc.vector.tensor_scalar(v[:, :, 1:2], su, subcol, None, OP.add)
                    return s2

                X2 = samples(wstart, swh[:, 0:1], "X2")
                Y2 = samples(hstart, swh[:, 1:2], "Y2")

                # per-axis: validity, clip, floor/frac, corner weight pairs,
                # and (for y only) the clamped corner index pair
                def axis_side(S2, lim, name, want_i4):
                    # valid = (S2 >= -0.5) & (S2 <= lim + 0.5)
                    va = wk.tile([R, 2 * NB], F32, tag=name + "va")
                    nc.vector.tensor_scalar(va[:, :], S2[:, :], -0.5, None, OP.is_ge)
                    vv = wk.tile([R, 2 * NB], F32, tag=name + "vv")
                    nc.vector.scalar_tensor_tensor(
                        vv[:, :], S2[:, :], lim + 0.5, va[:, :], OP.is_le, OP.mult
                    )
                    cl = wk.tile([R, 2 * NB], F32, tag=name + "cl")
                    nc.vector.tensor_scalar(cl[:, :], S2[:, :], 0.0, lim, OP.max, OP.min)
                    flo, fra = _floor(nc, wk, cl[:, :], name + "fl")
                    # count over the 2 samples, per bin -> reciprocal (1 or .5)
                    cnt = wk.tile([R, NB], F32, tag=name + "ct")
                    vvv = vv[:, :].rearrange("p (b s) -> p b s", s=2)
                    nc.vector.tensor_tensor(
                        cnt[:, :].unsqueeze(2),
                        vvv[:, :, 0:1], vvv[:, :, 1:2], OP.add,
                    )
                    eq2 = wk.tile([R, NB], F32, tag=name + "e2")
                    nc.vector.tensor_scalar(eq2[:, :], cnt[:, :], 2.0, None, OP.is_equal)
                    rc = wk.tile([R, NB], F32, tag=name + "rc")
                    nc.vector.tensor_scalar(rc[:, :], eq2[:, :], -0.5, 1.0, OP.mult, OP.add)
                    # weight pair: w0 = v*(1-f)*rc, w1 = v*f*rc  [R, 196] = (b, s, c)
                    rcb = rc[:, :].unsqueeze(2).broadcast_to([R, NB, 2])
                    vr = wk.tile([R, 2 * NB], F32, tag=name + "vr")
                    nc.vector.tensor_tensor(
                        vr[:, :].rearrange("p (b s) -> p b s", s=2), vvv, rcb, OP.mult
                    )
                    w1 = wk.tile([R, 2 * NB], F32, tag=name + "w1")
                    nc.vector.tensor_tensor(w1[:, :], vr[:, :], fra, OP.mult)
                    w0 = wk.tile([R, 2 * NB], F32, tag=name + "w0")
                    nc.vector.tensor_tensor(w0[:, :], vr[:, :], w1[:, :], OP.subtract)
                    W4 = wk.tile([R, 4 * NB], F32, tag=name + "W4")
                    W4v = W4[:, :].rearrange("p (b s c) -> p b s c", s=2, c=2)
                    w0v = w0[:, :].rearrange("p (b s) -> p b s", s=2).unsqueeze(3)
                    w1v = w1[:, :].rearrange("p (b s) -> p b s", s=2).unsqueeze(3)
                    nc.vector.tensor_copy(W4v[:, :, :, 0:1], w0v)
                    nc.vector.tensor_copy(W4v[:, :, :, 1:2], w1v)
                    if not want_i4:
                        return W4, None, flo
                    # index pair: i0 = floor, i1 = min(floor+1, lim)
                    I4 = wk.tile([R, 4 * NB], F32, tag=name + "I4")
                    I4v = I4[:, :].rearrange("p (b s c) -> p b s c", s=2, c=2)
                    flv = flo.rearrange("p (b s) -> p b s", s=2).unsqueeze(3)
                    nc.vector.tensor_copy(I4v[:, :, :, 0:1], flv)
                    nc.vector.tensor_scalar(I4v[:, :, :, 1:2], flv, 1.0, lim, OP.add, OP.min)
                    return W4, I4, flo

                WX4, _, XFL = axis_side(X2, float(W - 1), "x", False)
                WY4, YI4, _ = axis_side(Y2, float(H - 1), "y", True)

                # y-side indices -> flat row base: b*H*W + y*W
                YIr = wk.tile([R, 4 * NB], F32)
                nc.vector.tensor_scalar(
                    YIr[:, :], YI4[:, :], float(W), ybase[:, :], OP.mult, OP.add
                )

                # weights expanded to desc terms, bin-major [R, 392] =
                # (b, h, y, s), one tensor per pixel half (left x1 / right x1+1)
                WX4p = WX4[:, :].rearrange("p (b s c) -> p b c s", s=2, c=2)
                WY4b = (
                    WY4[:, :].rearrange("p (b h y) -> p b h y", h=2, y=2)
                    .unsqueeze(4).broadcast_to([R, NB, 2, 2, 2])
                )
                WtL = wk.tile([R, TD], F32, tag="WtL")
                WtR = wk.tile([R, TD], F32, tag="WtR")
                for Wh, f in ((WtL, 0), (WtR, 1)):
                    Whv = Wh[:, :].rearrange(
                        "p (b h y s) -> p b h y s", h=2, y=2, s=2
                    )
                    nc.vector.tensor_copy(Whv[:, :, :, :, :], WY4b)
                    wxf = WX4p[:, :, f : f + 1, :].unsqueeze(2)
                    for j in range(4):
                        h, y = j >> 1, j & 1
                        dstW = Whv[:, :, h : h + 1, y : y + 1, :]
                        nc.vector.tensor_tensor(dstW, dstW, wxf, OP.mult)

                # descriptor indices lane-major [R, 392] = (h, y, s, b):
                # idx = b*H*W + y_corner*W + floor(x_sample)
                IDX3 = wk.tile([R, TD], F32)
                IDX3v = IDX3[:, :].rearrange(
                    "p (h y s b) -> p h y s b", h=2, y=2, s=2
                )
                YIr2 = (
                    YIr[:, :].rearrange("p (b h y) -> p h y b", h=2, y=2)
                    .unsqueeze(3).broadcast_to([R, 2, 2, 2, NB])
                )
                nc.vector.tensor_copy(IDX3v[:, :, :, :, :], YIr2)
                XFL2 = (
                    XFL.rearrange("p (b s) -> p s b", s=2)
                    .unsqueeze(1).unsqueeze(2)
                )
                for j in range(4):
                    h, y = j >> 1, j & 1
                    dstI = IDX3v[:, h : h + 1, y : y + 1, :, :]
                    nc.vector.tensor_tensor(dstI, dstI, XFL2, OP.add)

                # ---- phase B: int16 gather indices in dma_gather's 16-lane
                # layout: desc i of pair q reads IDXG[i%16, 49q + i//16];
                # i = 16b + 2*t3 + r -> lane 2*t3+r, col = bin.
                IDX16 = wk.tile([R, TD], I16)
                nc.vector.tensor_copy(IDX16[:, :], IDX3[:, :])
                IDXG = wk.tile([128, NPAIR * NB], I16)
                for t3 in range(T3):
                    for r in range(2):
                        nc.sync.dma_start(
                            IDXG[2 * t3 + r : 2 * t3 + r + 1, :],
                            IDX16[32 * r : 32 * r + 32, t3 * NB : (t3 + 1) * NB],
                        )
                # Q7 tx/rx cpus each read their own 16-partition window of the
                # index tensor -> replicate lane group 0 across all 8 groups.
                for grp in range(1, 8):
                    nc.sync.dma_start(
                        IDXG[16 * grp : 16 * (grp + 1), :], IDXG[0:16, :]
                    )

                # Weights at gather partitions: desc d = 128c + p with
                # p = 2*(8*(b%8) + t3) + r, so transposing WtL/WtR in 64-wide
                # windows (j = 8*(b%8)+t3 contiguous per column) and writing
                # with one stride-2-partition DMA per (half, roi-half) puts
                # weight j at partition 2j + r.  Staged per (half, r) so it
                # is 4 DMAs total.
                WL = wk.tile([128, NCOL * NPAIR], F32, tag="WL")
                WR = wk.tile([128, NCOL * NPAIR], F32, tag="WR")
                nc.vector.memset(WL[:, :], 0.0)
                nc.vector.memset(WR[:, :], 0.0)
                for Wh, Wsrc, tag in ((WL, WtL, "L"), (WR, WtR, "R")):
                    st0 = wk.tile([64, NCOL * NPAIR], F32, tag="st0" + tag)
                    st1 = wk.tile([64, NCOL * NPAIR], F32, tag="st1" + tag)
                    # col 6 rows 8.. (-> Wh partitions 16..) never gathered:
                    # keep finite zeros
                    nc.vector.memset(st0[:, 6 * NPAIR :], 0.0)
                    nc.vector.memset(st1[:, 6 * NPAIR :], 0.0)
                    for c in range(NCOL):
                        n = 64 if c < 6 else 8
                        ps = pst.tile([64, R], F32, tag="pstr")
                        nc.tensor.transpose(
                            ps[0:n, :], Wsrc[:, 64 * c : 64 * c + n], iden[:, :]
                        )
                        nc.vector.tensor_copy(
                            st0[0:n, c * NPAIR : (c + 1) * NPAIR], ps[0:n, 0:32]
                        )
                        nc.vector.tensor_copy(
                            st1[0:n, c * NPAIR : (c + 1) * NPAIR], ps[0:n, 32:64]
                        )
                    nc.sync.dma_start(Wh[0::2, :], st0[:, :])
                    nc.sync.dma_start(Wh[1::2, :], st1[:, :])

                # ---- phase C: gather + weighted reduce, one pair at a time --
                out_v = out_d.ap().rearrange("r (b c) -> b r c", c=CH)
                for q in range(NPAIR):
                    gt = gp.tile([128, NCOL * EL], BF16)
                    if q < GP_BUFS or bench_mode == 2:
                        # col 6 partitions 16.. are never gathered; clear on
                        # first use so masked-0 products stay finite.
                        nc.vector.memset(gt[:, 6 * EL : 7 * EL], 0.0)
                    if bench_mode != 2:
                        dest = gt[:, :].rearrange("p (j f) -> p j f", f=EL)
                        nc.gpsimd.dma_gather(
                            dest,
                            dap,
                            IDXG[:, q * NB : (q + 1) * NB],
                            ND,
                            ND,
                            EL,
                            elem_step=CH,
                            queue_num=q % 4,
                        )
                    if bench_mode == 1:
                        continue
                    # weighted masks (bf16) for left / right pixels
                    wmL = gwp.tile([128, NCOL * M2], BF16, tag="wmL")
                    wmR = gwp.tile([128, NCOL * M2], BF16, tag="wmR")
                    for wm, wsrc in ((wmL, WL), (wmR, WR)):
                        wtb = (
                            wsrc[:, :].rearrange("p (c q) -> p c q", q=NPAIR)
                            [:, :, q : q + 1].broadcast_to([128, NCOL, M2])
                        )
                        nc.any.tensor_tensor(
                            wm[:, :].rearrange("p (c j) -> p c j", j=M2),
                            cm[:, :].rearrange("p (c j) -> p c j", j=M2),
                            wtb,
                            OP.mult,
                        )
                    ps = psp.tile([M2, CH], F32)
                    for c in range(NCOL):
                        nc.tensor.matmul(
                            ps[:, :],
                            wmL[:, c * M2 : (c + 1) * M2],
                            gt[:, c * EL : c * EL + CH],
                            start=(c == 0),
                            stop=False,
                        )
                        nc.tensor.matmul(
                            ps[:, :],
                            wmR[:, c * M2 : (c + 1) * M2],
                            gt[:, c * EL + CH : (c + 1) * EL],
                            start=False,
                            stop=(c == NCOL - 1),
                        )
                    # outputs batched 4 pairs per DMA (rows q0..q0+3 and
                    # q0+32..q0+35)
                    if q % 4 == 0:
                        ob = obp.tile([M2, 4 * CH], F32, tag="ob")
                    k = q % 4
                    nc.scalar.copy(ob[:, k * CH : (k + 1) * CH], ps[:, :])
                    if q % 4 == 3:
                        q0 = q - 3
                        nc.sync.dma_start(
                            out_v[:, q0 : q0 + 4, :],
                            ob[0:NB, :].rearrange("p (r c) -> p r c", c=CH),
                        )
                        nc.sync.dma_start(
                            out_v[:, q0 + 32 : q0 + 36, :],
                            ob[NB:M2, :].rearrange("p (r c) -> p r c", c=CH),
                        )

    nc.finalize()
    return nc


def host_constants():
    iopw = np.tile((np.arange(NB) % P).astype(np.float32), (R, 1))
    ioph = np.tile((np.arange(NB) // P).astype(np.float32), (R, 1))
    iden = np.eye(R, dtype=np.float32)
    # mask from the descriptor stream: desc i = 16b + 2*t3 + r
    cm = np.zeros((128, NCOL * M2), dtype=np.float32)
    for b in range(NB):
        for t3 in range(T3):
            for r in range(2):
                i = 16 * b + 2 * t3 + r
                cm[i % 128, (i // 128) * M2 + NB * r + b] = 1.0
    return {"iota_pw": iopw, "iota_ph": ioph, "identity": iden, "cmask": cm}


_cache = {}


def _program():
    if "nc" not in _cache:
        _cache["nc"] = build_program()
    return _cache["nc"]


def run(data, rois, offset, **spmd_kwargs):
    data = np.asarray(data, dtype=np.float32)
    rois = np.asarray(rois, dtype=np.float32)
    offset = np.asarray(offset, dtype=np.float32)
    n_rois = rois.shape[0]
    data_t = (
        np.ascontiguousarray(data.transpose(0, 2, 3, 1))
        .reshape(NPX, CH)
        .astype(ml_dtypes.bfloat16)
    )
    data_t = np.concatenate(
        [data_t, np.zeros((PAD, CH), dtype=ml_dtypes.bfloat16)], axis=0
    )
    consts = host_constants()
    in_maps = []
    for c in range(N_CORES):
        sl = slice(c * R, (c + 1) * R)
        m = {
            "data_t": data_t,
            "rois": rois[sl],
            "offs": offset[sl].reshape(R, 2 * NB),
        }
        m.update(consts)
        in_maps.append(m)
    res = run_bass_kernel_spmd(
        _program(), in_maps, core_ids=list(range(N_CORES)), **spmd_kwargs
    )
    outs = np.concatenate([res.results[c]["out"] for c in range(N_CORES)], axis=0)
    out = outs.reshape(n_rois, NB, CH).transpose(0, 2, 1).reshape(n_rois, CH, P, P)
    return np.ascontiguousarray(out), res


def kernel(data, rois, offset):
    out, _ = run(data, rois, offset)
    return out



# revision 4
# speedup vs baseline: 1.0158x; 1.0158x over previous
"""Deformable PS-ROI pooling on Trainium2 (Bass/Tile), SPMD over 8 cores.

v2: same gather/reduce architecture as the baseline (2-px descriptors, pair
gathers, mask matmul reduce) with the ramp-to-first-gather collapsed:

- The index-critical chain (roi coords -> sample floors -> flat gather
  indices) is emitted first, alone, on DVE, using plain f32->int converts
  (truncation == floor for the non-negative clipped coordinates; off-by-one
  cases at exact integers are compensated exactly by the bilinear weights).
- The 16 idx lane DMAs + 7 replication copies alternate between the SP and
  Activation HWDGE queues instead of serializing on SP.
- The weight chain / PE transposes / mask staging all happen after the
  gather stream has been unblocked; gathers only need IDXG.
- Output is written as bf16 (halves out-DMA bytes; tolerance is 2e-2).
"""

import numpy as np
import ml_dtypes

import concourse.bass as bass
import concourse.bacc as bacc
import concourse.mybir as mybir
from concourse import tile
from concourse.bass_utils import run_bass_kernel_spmd

F32 = mybir.dt.float32
BF16 = mybir.dt.bfloat16
I32 = mybir.dt.int32
I16 = mybir.dt.int16
OP = mybir.AluOpType

N_CORES = 8
R = 64                  # rois per core
P = 7                   # pooled output size
NB = P * P              # 49 bins
CH = 256                # channels
H = W = 128             # feature map spatial
B = 2                   # batch
NPX = B * H * W         # 32768 flat pixels
PAD = 4                 # extra zero pixels (right-px overrun at x1=W-1)
T3 = 8                  # terms per (bin, roi): sample_h x corner_y x sample_w
TD = NB * T3            # 392 descriptor-terms per roi
ND = 2 * TD             # 784 descriptors per pair
NPAIR = R // 2
NCOL = 7                # gather dest cols per pair (784 = 6*128 + 16)
EL = 2 * CH             # elements per descriptor (2 pixels)
M2 = 2 * NB             # 98 psum rows per pair
SCALE = 0.0625
TRANS_STD = 0.1
GP_BUFS = 8


def build_program(reps: int = 1, bench_mode: int = 0, sim_safe: bool = False):
    """bench_mode: 0=full kernel, 1=gathers only, 2=no gathers, 3=A/B only.
    sim_safe adds memsets that only exist to appease the exec-simulator's
    uninitialized-memory tracker (strided-partition DMA writes)."""
    nc = bacc.Bacc("TRN2", target_bir_lowering=False, debug=False, num_swdge_queues=4)
    nc.dynamic_dma_scratch_size = 2 ** 16

    data = nc.dram_tensor("data_t", [NPX + PAD, CH], BF16, kind="ExternalInput")
    scr2_d = nc.dram_tensor("scr2", [R, TD], I16, kind="Internal")
    scr8_d = nc.dram_tensor("scr8", [128, NPAIR * NB], I16, kind="Internal")
    rois_d = nc.dram_tensor("rois", [R, 5], F32, kind="ExternalInput")
    off_d = nc.dram_tensor("offs", [R, 2 * NB], F32, kind="ExternalInput")
    iopw_d = nc.dram_tensor("iota_pw", [R, NB], F32, kind="ExternalInput")
    ioph_d = nc.dram_tensor("iota_ph", [R, NB], F32, kind="ExternalInput")
    iden_d = nc.dram_tensor("identity", [R, R], F32, kind="ExternalInput")
    cm_d = nc.dram_tensor("cmask", [128, NCOL * M2], F32, kind="ExternalInput")
    out_d = nc.dram_tensor("out", [R, NB * CH], BF16, kind="ExternalOutput")

    with tile.TileContext(nc) as tc:
        with (
            tc.tile_pool(name="const", bufs=1) as cst,
            tc.tile_pool(name="work", bufs=1) as wk,
            tc.tile_pool(name="gwp", bufs=8) as gwp,
            tc.tile_pool(name="obp", bufs=3) as obp,
            tc.tile_pool(name="psp", bufs=4, space="PSUM") as psp,
            tc.tile_pool(name="pst", bufs=2, space="PSUM") as pst,
        ):
            # ---- inputs: index-critical ones (rois, iotas, off) first on SP,
            # the reduce-side constants (iden, cm) on Act ----
            rois = cst.tile([R, 5], F32)
            nc.sync.dma_start(rois[:, :], rois_d.ap())
            iopw = cst.tile([R, NB], F32)
            nc.sync.dma_start(iopw[:, :], iopw_d.ap())
            ioph = cst.tile([R, NB], F32)
            nc.sync.dma_start(ioph[:, :], ioph_d.ap())
            off = cst.tile([R, 2 * NB], F32)
            nc.sync.dma_start(off[:, :], off_d.ap())
            iden = cst.tile([R, R], F32)
            nc.scalar.dma_start(iden[:, :], iden_d.ap())
            cm = cst.tile([128, NCOL * M2], F32)
            nc.scalar.dma_start(cm[:, :], cm_d.ap())

            # gather buffer ring; clear the never-gathered col-6 tail once
            # (on Pool, which is idle until the first gather)
            gts = []
            for i in range(GP_BUFS):
                gt_i = cst.tile([128, NCOL * EL], BF16, tag=f"gt{i}")
                gts.append(gt_i)
            for g in gts:
                nc.gpsimd.memset(g[:, 6 * EL : 7 * EL], 0.0)

            # gather source: 2 contiguous pixels per desc, row stride 1 pixel
            a0 = data.ap()
            dap = bass.AP(a0.tensor, a0.offset, [[CH, NPX], [1, EL]])

            from contextlib import nullcontext
            # bench_mode >= 4: hoist phase A/B out of the rep loop so the
            # loop body is phase C only (4 = gather+reduce, 5 = gathers only)
            hoist = bench_mode >= 4
            loop_cm = (tc.For_i(0, reps, 1) if reps > 1 and not hoist
                       else nullcontext())
            with loop_cm:
                V = nc.vector
                # ============ index-critical chain (DVE only) ============
                # rnd = floor(rois[:,1:5] + 0.5) == RTE-convert on hardware
                # (differs only at exact half-integers, measure-zero inputs)
                rndi = wk.tile([R, 4], I32, tag="rndi")
                V.tensor_copy(rndi[:, :], rois[:, 1:5])
                rnd = wk.tile([R, 4], F32, tag="rnd")
                V.tensor_copy(rnd[:, :], rndi[:, :])

                swsh = wk.tile([R, 2], F32, tag="swsh")
                V.tensor_scalar(swsh[:, :], rnd[:, 0:2], SCALE, -0.5, OP.mult, OP.add)
                eweh = wk.tile([R, 2], F32, tag="eweh")
                V.tensor_scalar(eweh[:, :], rnd[:, 2:4], SCALE, SCALE - 0.5,
                                OP.mult, OP.add)
                rwh0 = wk.tile([R, 2], F32, tag="rwh0")
                V.tensor_tensor(rwh0[:, :], eweh[:, :], swsh[:, :], OP.subtract)
                rwh = wk.tile([R, 2], F32, tag="rwh")
                V.tensor_scalar(rwh[:, :], rwh0[:, :], 0.1, None, OP.max)
                bwh = wk.tile([R, 2], F32, tag="bwh")
                V.tensor_scalar(bwh[:, :], rwh[:, :], 1.0 / P, None, OP.mult)
                swh = wk.tile([R, 2], F32, tag="swh")
                V.tensor_scalar(swh[:, :], bwh[:, :], 0.5, None, OP.mult)
                rwh01 = wk.tile([R, 2], F32, tag="rwh01")
                V.tensor_scalar(rwh01[:, :], rwh[:, :], TRANS_STD, None, OP.mult)
                ybase = wk.tile([R, 1], F32, tag="ybase")
                V.tensor_scalar(ybase[:, :], rois[:, 0:1], float(H * W), None, OP.mult)

                # bin starts [R, 49]: iota*b + (off*r01 + s)
                def bin_start(iota, bcol, scol, tview, r01col, name):
                    t0 = wk.tile([R, NB], F32, tag=name + "0")
                    V.tensor_scalar(t0[:, :], iota, bcol, None, OP.mult)
                    t1 = wk.tile([R, NB], F32, tag=name + "1")
                    V.scalar_tensor_tensor(t1[:, :], tview, r01col, t0[:, :],
                                           OP.mult, OP.add)
                    t2 = wk.tile([R, NB], F32, tag=name + "2")
                    V.tensor_scalar(t2[:, :], t1[:, :], scol, None, OP.add)
                    return t2

                wstart = bin_start(iopw[:, :], bwh[:, 0:1], swsh[:, 0:1],
                                   off[:, 0:NB], rwh01[:, 0:1], "ws")
                hstart = bin_start(ioph[:, :], bwh[:, 1:2], swsh[:, 1:2],
                                   off[:, NB : 2 * NB], rwh01[:, 1:2], "hs")

                # sample positions [R, 98] = (bin, s)
                def samples(start, subcol, name):
                    s2 = wk.tile([R, 2 * NB], F32, tag=name)
                    v = s2[:, :].rearrange("p (b s) -> p b s", s=2)
                    su = start[:, :].unsqueeze(2)
                    V.tensor_copy(v[:, :, 0:1], su)
                    V.tensor_scalar(v[:, :, 1:2], su, subcol, None, OP.add)
                    return s2

                X2 = samples(wstart, swh[:, 0:1], "X2")
                Y2 = samples(hstart, swh[:, 1:2], "Y2")

                # clip / floor per axis.  HW converts f32->i32 with RTE, so
                # convert(x - 0.5) == floor(x) except at exact integers where
                # it can be floor(x)-1 with frac == 1.0 -- which the bilinear
                # weights compensate exactly (and stays in-bounds).
                def clip_floor(S2, lim, name):
                    cl = wk.tile([R, 2 * NB], F32, tag=name + "cl")
                    V.tensor_scalar(cl[:, :], S2[:, :], 0.0, lim, OP.max, OP.min)
                    fi = wk.tile([R, 2 * NB], I32, tag=name + "fi")
                    V.tensor_scalar(fi[:, :], cl[:, :], 0.5, None, OP.subtract)
                    fl = wk.tile([R, 2 * NB], F32, tag=name + "fl")
                    V.tensor_copy(fl[:, :], fi[:, :])
                    return cl, fl

                Xcl, XFL = clip_floor(X2, float(W - 1), "x")
                Ycl, YFL = clip_floor(Y2, float(H - 1), "y")

                # y corner rows [R, 4NB] = (b, h, y): y0 = fl, y1 = min(fl+1, 127)
                YI4 = wk.tile([R, 4 * NB], F32, tag="YI4")
                YI4v = YI4[:, :].rearrange("p (b h y) -> p b h y", h=2, y=2)
                YFLv = YFL[:, :].rearrange("p (b h) -> p b h", h=2).unsqueeze(3)
                V.tensor_copy(YI4v[:, :, :, 0:1], YFLv)
                V.tensor_scalar(YI4v[:, :, :, 1:2], YFLv, 1.0, float(H - 1),
                                OP.add, OP.min)
                # flat row base: b*H*W + y*W
                YIr = wk.tile([R, 4 * NB], F32, tag="YIr")
                V.tensor_scalar(YIr[:, :], YI4[:, :], float(W), ybase[:, :],
                                OP.mult, OP.add)

                # descriptor indices lane-major [R, 392] = (hy, s, b)
                IDX3 = wk.tile([R, TD], F32, tag="IDX3")
                IDX3v = IDX3[:, :].rearrange("p (hy s b) -> p hy s b", hy=4, s=2)
                YIr2 = (YIr[:, :].rearrange("p (b hy) -> p hy b", hy=4)
                        .unsqueeze(2).broadcast_to([R, 4, 2, NB]))
                V.tensor_copy(IDX3v[:, :, :, :], YIr2)
                XFL2 = (XFL[:, :].rearrange("p (b s) -> p s b", s=2)
                        .unsqueeze(1).broadcast_to([R, 4, 2, NB]))
                V.tensor_tensor(IDX3v[:, :, :, :], IDX3v[:, :, :, :], XFL2, OP.add)
                IDX16 = wk.tile([R, TD], I16, tag="IDX16")
                V.tensor_copy(IDX16[:, :], IDX3[:, :])

                # ============ IDXG staging via DRAM bounce (9 DMAs) ============
                # A: dump IDX16 to DRAM; B/C: strided-partition shuffle
                # readbacks (one per roi half) into lane layout; D: dump the
                # 16-lane group; E/F/G: DRAM-side doubling to 8 groups;
                # H1/H2: bulk readback in column halves so the first 16
                # gathers start before the second half lands.
                IDXG = wk.tile([128, NPAIR * NB], I16, tag="IDXG")
                if sim_safe:
                    nc.vector.memset(IDXG[:, :], 0)
                nc.sync.dma_start(scr2_d.ap(), IDX16[:, :])
                for r in range(2):
                    src = scr2_d.ap()[32 * r : 32 * r + 32, :].rearrange(
                        "q (t b) -> t q b", t=T3)
                    eng = nc.sync if r == 0 else nc.scalar
                    eng.dma_start(IDXG[r : 16 : 2, :], src)
                nc.sync.dma_start(scr8_d.ap()[0:16, :], IDXG[0:16, :])
                for grp in range(1, 8):
                    eng = nc.sync if grp % 2 == 1 else nc.scalar
                    eng.dma_start(scr8_d.ap()[16 * grp : 16 * (grp + 1), :],
                                  scr8_d.ap()[0:16, :])
                HALF = (NPAIR // 2) * NB
                nc.sync.dma_start(IDXG[16:128, 0:HALF],
                                  scr8_d.ap()[16:128, 0:HALF])
                nc.scalar.dma_start(IDXG[16:128, HALF:],
                                    scr8_d.ap()[16:128, HALF:])

                # ============ weight chain (DVE, after IDXG emits) ============
                # validity from unclipped sample positions
                def weights_axis(S2, cl, fl, lim, name):
                    va = wk.tile([R, 2 * NB], F32, tag=name + "va")
                    V.tensor_scalar(va[:, :], S2[:, :], -0.5, None, OP.is_ge)
                    vv = wk.tile([R, 2 * NB], F32, tag=name + "vv")
                    V.scalar_tensor_tensor(vv[:, :], S2[:, :], lim + 0.5, va[:, :],
                                           OP.is_le, OP.mult)
                    fra = wk.tile([R, 2 * NB], F32, tag=name + "fr")
                    V.tensor_tensor(fra[:, :], cl[:, :], fl[:, :], OP.subtract)
                    cnt = wk.tile([R, NB], F32, tag=name + "ct")
                    vvv = vv[:, :].rearrange("p (b s) -> p b s", s=2)
                    V.tensor_tensor(cnt[:, :].unsqueeze(2), vvv[:, :, 0:1],
                                    vvv[:, :, 1:2], OP.add)
                    eq2 = wk.tile([R, NB], F32, tag=name + "e2")
                    V.tensor_scalar(eq2[:, :], cnt[:, :], 2.0, None, OP.is_equal)
                    rc = wk.tile([R, NB], F32, tag=name + "rc")
                    V.tensor_scalar(rc[:, :], eq2[:, :], -0.5, 1.0, OP.mult, OP.add)
                    rcb = rc[:, :].unsqueeze(2).broadcast_to([R, NB, 2])
                    vr = wk.tile([R, 2 * NB], F32, tag=name + "vr")
                    V.tensor_tensor(vr[:, :].rearrange("p (b s) -> p b s", s=2),
                                    vvv, rcb, OP.mult)
                    w1 = wk.tile([R, 2 * NB], F32, tag=name + "w1")
                    V.tensor_tensor(w1[:, :], vr[:, :], fra[:, :], OP.mult)
                    w0 = wk.tile([R, 2 * NB], F32, tag=name + "w0")
                    V.tensor_tensor(w0[:, :], vr[:, :], w1[:, :], OP.subtract)
                    W4 = wk.tile([R, 4 * NB], F32, tag=name + "W4")
                    W4v = W4[:, :].rearrange("p (b s c) -> p b s c", s=2, c=2)
                    w0v = w0[:, :].rearrange("p (b s) -> p b s", s=2).unsqueeze(3)
                    w1v = w1[:, :].rearrange("p (b s) -> p b s", s=2).unsqueeze(3)
                    V.tensor_copy(W4v[:, :, :, 0:1], w0v)
                    V.tensor_copy(W4v[:, :, :, 1:2], w1v)
                    return W4

                WX4 = weights_axis(X2, Xcl, XFL, float(W - 1), "x")
                WY4 = weights_axis(Y2, Ycl, YFL, float(H - 1), "y")

                # weights expanded to desc terms, bin-major [R, 392] =
                # (b, h, y, s), one tensor per pixel half
                WX4p = WX4[:, :].rearrange("p (b s c) -> p b c s", s=2, c=2)
                WY4b = (WY4[:, :].rearrange("p (b h y) -> p b h y", h=2, y=2)
                        .unsqueeze(4).broadcast_to([R, NB, 2, 2, 2]))
                WtL = wk.tile([R, TD], F32, tag="WtL")
                WtR = wk.tile([R, TD], F32, tag="WtR")
                for Wh, f in ((WtL, 0), (WtR, 1)):
                    Whv = Wh[:, :].rearrange("p (b h y s) -> p b h y s", h=2, y=2, s=2)
                    V.tensor_copy(Whv[:, :, :, :, :], WY4b)
                    wxf = WX4p[:, :, f : f + 1, :].unsqueeze(2)
                    for j in range(4):
                        h, y = j >> 1, j & 1
                        dstW = Whv[:, :, h : h + 1, y : y + 1, :]
                        V.tensor_tensor(dstW, dstW, wxf, OP.mult)

                # ---- weight staging to gather partitions (PE transposes +
                # stride-2 partition DMAs), off the gather critical path ----
                WL = wk.tile([128, NCOL * NPAIR], F32, tag="WL")
                WR = wk.tile([128, NCOL * NPAIR], F32, tag="WR")
                nc.vector.memset(WL[:, :], 0.0)
                nc.vector.memset(WR[:, :], 0.0)
                for Wh, Wsrc, tag in ((WL, WtL, "L"), (WR, WtR, "R")):
                    st0 = wk.tile([64, NCOL * NPAIR], F32, tag="st0" + tag)
                    st1 = wk.tile([64, NCOL * NPAIR], F32, tag="st1" + tag)
                    nc.vector.memset(st0[:, 6 * NPAIR :], 0.0)
                    nc.vector.memset(st1[:, 6 * NPAIR :], 0.0)
                    for c in range(NCOL):
                        n = 64 if c < 6 else 8
                        ps = pst.tile([64, R], F32, tag="pstr")
                        nc.tensor.transpose(
                            ps[0:n, :], Wsrc[:, 64 * c : 64 * c + n], iden[:, :]
                        )
                        nc.vector.tensor_copy(
                            st0[0:n, c * NPAIR : (c + 1) * NPAIR], ps[0:n, 0:32]
                        )
                        nc.vector.tensor_copy(
                            st1[0:n, c * NPAIR : (c + 1) * NPAIR], ps[0:n, 32:64]
                        )
                    eng = nc.sync if tag == "L" else nc.scalar
                    eng.dma_start(Wh[0::2, :], st0[:, :])
                    eng.dma_start(Wh[1::2, :], st1[:, :])

                # ============ phase C: gather + weighted reduce ============
                loop_cm2 = (tc.For_i(0, reps, 1) if reps > 1 and hoist
                            else nullcontext())
                loop_cm2.__enter__()
                out_v = out_d.ap().rearrange("r (b c) -> b r c", c=CH)
                npair_run = 0 if bench_mode == 3 else NPAIR
                for q in range(npair_run):
                    gt = gts[q % GP_BUFS]
                    if bench_mode != 2:
                        dest = gt[:, :].rearrange("p (j f) -> p j f", f=EL)
                        nc.gpsimd.dma_gather(
                            dest,
                            dap,
                            IDXG[:, q * NB : (q + 1) * NB],
                            ND,
                            ND,
                            EL,
                            elem_step=CH,
                            queue_num=q % 4,
                        )
                    if bench_mode in (1, 5):
                        continue
                    # weighted masks (bf16) for left / right pixels
                    wmL = gwp.tile([128, NCOL * M2], BF16, tag="wmL")
                    wmR = gwp.tile([128, NCOL * M2], BF16, tag="wmR")
                    for wm, wsrc in ((wmL, WL), (wmR, WR)):
                        wtb = (
                            wsrc[:, :].rearrange("p (c q) -> p c q", q=NPAIR)
                            [:, :, q : q + 1].broadcast_to([128, NCOL, M2])
                        )
                        nc.any.tensor_tensor(
                            wm[:, :].rearrange("p (c j) -> p c j", j=M2),
                            cm[:, :].rearrange("p (c j) -> p c j", j=M2),
                            wtb,
                            OP.mult,
                        )
                    ps = psp.tile([M2, CH], F32)
                    for c in range(NCOL):
                        nc.tensor.matmul(
                            ps[:, :],
                            wmL[:, c * M2 : (c + 1) * M2],
                            gt[:, c * EL : c * EL + CH],
                            start=(c == 0),
                            stop=False,
                        )
                        nc.tensor.matmul(
                            ps[:, :],
                            wmR[:, c * M2 : (c + 1) * M2],
                            gt[:, c * EL + CH : (c + 1) * EL],
                            start=False,
                            stop=(c == NCOL - 1),
                        )
                    if q % 4 == 0:
                        ob = obp.tile([M2, 4 * CH], BF16, tag="ob")
                    k = q % 4
                    nc.scalar.copy(ob[:, k * CH : (k + 1) * CH], ps[:, :])
                    if q % 4 == 3:
                        q0 = q - 3
                        nc.sync.dma_start(
                            out_v[:, q0 : q0 + 4, :],
                            ob[0:NB, :].rearrange("p (r c) -> p r c", c=CH),
                        )
                        nc.sync.dma_start(
                            out_v[:, q0 + 32 : q0 + 36, :],
                            ob[NB:M2, :].rearrange("p (r c) -> p r c", c=CH),
                        )
                loop_cm2.__exit__(None, None, None)

    nc.finalize()
    return nc


def host_constants():
    iopw = np.tile((np.arange(NB) % P).astype(np.float32), (R, 1))
    ioph = np.tile((np.arange(NB) // P).astype(np.float32), (R, 1))
    iden = np.eye(R, dtype=np.float32)
    # mask from the descriptor stream: desc i = 16b + 2*t3 + r
    cm = np.zeros((128, NCOL * M2), dtype=np.float32)
    for b in range(NB):
        for t3 in range(T3):
            for r in range(2):
                i = 16 * b + 2 * t3 + r
                cm[i % 128, (i // 128) * M2 + NB * r + b] = 1.0
    return {"iota_pw": iopw, "iota_ph": ioph, "identity": iden, "cmask": cm}


_cache = {}


def _program():
    if "nc" not in _cache:
        _cache["nc"] = build_program()
    return _cache["nc"]


def build_in_maps(data, rois, offset):
    data = np.asarray(data, dtype=np.float32)
    rois = np.asarray(rois, dtype=np.float32)
    offset = np.asarray(offset, dtype=np.float32)
    data_t = (
        np.ascontiguousarray(data.transpose(0, 2, 3, 1))
        .reshape(NPX, CH)
        .astype(ml_dtypes.bfloat16)
    )
    data_t = np.concatenate(
        [data_t, np.zeros((PAD, CH), dtype=ml_dtypes.bfloat16)], axis=0
    )
    consts = host_constants()
    in_maps = []
    for c in range(N_CORES):
        sl = slice(c * R, (c + 1) * R)
        m = {
            "data_t": data_t,
            "rois": rois[sl],
            "offs": offset[sl].reshape(R, 2 * NB),
        }
        m.update(consts)
        in_maps.append(m)
    return in_maps


def run(data, rois, offset, **spmd_kwargs):
    n_rois = np.asarray(rois).shape[0]
    in_maps = build_in_maps(data, rois, offset)
    res = run_bass_kernel_spmd(
        _program(), in_maps, core_ids=list(range(N_CORES)), **spmd_kwargs
    )
    outs = np.concatenate(
        [np.asarray(res.results[c]["out"], dtype=np.float32)
         for c in range(N_CORES)], axis=0
    )
    out = outs.reshape(n_rois, NB, CH).transpose(0, 2, 1).reshape(n_rois, CH, P, P)
    return np.ascontiguousarray(out), res


def kernel(data, rois, offset):
    out, _ = run(data, rois, offset)
    return out
